# revision 19
# baseline (speedup 1.0000x reference)
"""Trainium2 Bass kernel for DiagonalS5SSM.

Math (per batch b, with the reference's where(valid,...) elided — valid is a
prefix mask in l and the output is masked by the same prefix, so the frozen
tail states never reach the output):

    it[l, n]  = sum_d x[b, l, d] * bbar[n, d]          (complex)
    s[l, n]   = abar[n] * s[l-1, n] + it[l, n]         (complex scan)
    y[b, l, :] = mask[l] * (Re(s[l] @ c^T) + x[b, l] @ D^T)

The complex scan is decoupled into two real scans via polar form
abar = rho * e^{i theta} (rho < 1, so no dynamic-range blowup):

    v[l] = e^{-i theta l} * it[l]       (elementwise rotation)
    w[l] = rho * w[l-1] + v[l]          (HW tensor_tensor_scan per plane)
    s[l] = e^{+i theta l} * w[l]        (rotation back)

Sharding: data-parallel over batch, 2 batches per core, no collectives.
Matmuls run as float32r (TF32-like, 1 cyc/row on the PE at free>=256).
x is pre-transposed on the host so the contraction dim lands on partitions
without PE transposes. The whole pipeline is chunk-streamed in 512-row
chunks; the scan chains across chunks via initial=prev[:, -1:].
"""

import numpy as np

import concourse.bass as bass
import concourse.tile as tile
from concourse import bacc, mybir
from concourse.bass_utils import run_bass_kernel_spmd

B, L, D, N = 16, 2048, 512, 256
NCORES = 8
BL = B // NCORES          # batches per core
R = BL * L                # rows per core (r = b*L + l)
NH = N // 128             # n-halves
DK = D // 128             # d-chunks
RT = R // 128             # 128-row tiles per core
RCH = R // 512            # 512-row chunks per core
CPB = L // 512            # chunks per batch

F32 = mybir.dt.float32
BF16 = mybir.dt.bfloat16
MM_DT = mybir.dt.float32r  # matmul operand dtype (float32r: 1 cyc/row)
EW_DT = BF16               # elementwise/rotation dtype (2x DVE mode)
S_DT = BF16                # s-plane / stage-5 matmul dtype

AluOp = mybir.AluOpType
ACT_COPY = mybir.ActivationFunctionType.Copy


def _bcast_cols(ap: bass.AP, n: int) -> bass.AP:
    """[128, 1] AP -> [128, n] free-broadcast (step-0) AP."""
    return bass.AP(tensor=ap.tensor, offset=ap.offset, ap=[ap.ap[0], [0, n]])


def build_nc():
    nc = bacc.Bacc(
        "TRN2",
        target_bir_lowering=False,
        debug=False,
        enable_asserts=False,
        num_devices=NCORES,
    )

    xt_d = nc.dram_tensor("xt", [D, R], MM_DT, kind="ExternalInput")
    xadd_d = nc.dram_tensor("xadd", [R, D], F32, kind="ExternalInput")
    w1_d = nc.dram_tensor("w1", [128, DK * 2 * NH * 128], MM_DT, kind="ExternalInput")
    w2_d = nc.dram_tensor("w2", [128, 2 * NH * D], S_DT, kind="ExternalInput")
    cos_d = nc.dram_tensor("cost", [128, NH * L], EW_DT, kind="ExternalInput")
    sin_d = nc.dram_tensor("sint", [128, NH * L], EW_DT, kind="ExternalInput")
    rho_d = nc.dram_tensor("rho", [128, NH], F32, kind="ExternalInput")
    mask_d = nc.dram_tensor("maskc", [128, RT], F32, kind="ExternalInput")
    y_d = nc.dram_tensor("y", [R, D], F32, kind="ExternalOutput")

    with tile.TileContext(nc) as tc:
        with (
            tc.tile_pool(name="consts", bufs=1) as consts,
            tc.tile_pool(name="wplanes", bufs=4) as wplanes,
            tc.tile_pool(name="xtp", bufs=6) as xt_p,
            tc.tile_pool(name="uvp", bufs=10) as uv_p,
            tc.tile_pool(name="sp", bufs=6) as s_p,
            tc.tile_pool(name="yp", bufs=3) as y_p,
            tc.tile_pool(name="ps_it", bufs=5, space="PSUM") as ps_it,
            tc.tile_pool(name="ps_y", bufs=3, space="PSUM") as ps_y,
        ):
            w1_sb = consts.tile([128, DK * 2 * NH * 128], MM_DT, tag="w1")
            for q in range(4):
                nc.sync.dma_start(w1_sb[q * 32:(q + 1) * 32, :],
                                  w1_d.ap()[q * 32:(q + 1) * 32, :])
            w2_sb = consts.tile([128, 2 * NH * D], S_DT, tag="w2")
            for q in range(2):
                nc.sync.dma_start(w2_sb[q * 64:(q + 1) * 64, :],
                                  w2_d.ap()[q * 64:(q + 1) * 64, :])
            cos_sb = consts.tile([128, NH * L], EW_DT, tag="cos")
            nc.gpsimd.dma_start(cos_sb[:], cos_d.ap())
            sin_sb = consts.tile([128, NH * L], EW_DT, tag="sin")
            nc.gpsimd.dma_start(sin_sb[:], sin_d.ap())
            rho_sb = consts.tile([128, NH], F32, tag="rho")
            nc.gpsimd.dma_start(rho_sb[:], rho_d.ap())
            mask_sb = consts.tile([128, RT], F32, tag="maskc")
            nc.gpsimd.dma_start(mask_sb[:], mask_d.ap())

            # full-width scan outputs, [n-half partitions, (b, l) free]
            w_pl = [
                [wplanes.tile([128, R], EW_DT, tag="wpl", name=f"w_{p}_{h}")
                 for h in range(NH)]
                for p in range(2)
            ]

            for rc in range(RCH):
                ccol = (rc % CPB) * 512   # l-offset within batch
                dst = slice(rc * 512, rc * 512 + 512)

                # x^T chunk [d, r] straight from DRAM (host pre-transposed)
                xt = []
                for k in range(DK):
                    t = xt_p.tile([128, 512], MM_DT, tag="xt", name=f"xt_{rc}_{k}")
                    nc.sync.dma_start(
                        t[:64, :], xt_d.ap()[k * 128:k * 128 + 64, dst]
                    )
                    nc.sync.dma_start(
                        t[64:, :], xt_d.ap()[k * 128 + 64:(k + 1) * 128, dst]
                    )
                    xt.append(t)

                # stage 1: it[n, r-chunk] = sum_d bbar[n, d] x[r, d]
                it_ps = [[None] * NH for _ in range(2)]
                for plane in range(2):
                    for half in range(NH):
                        ps = ps_it.tile([128, 512], F32, tag="it",
                                        name=f"it_{rc}_{plane}_{half}")
                        for k in range(DK):
                            col = ((k * 2 + plane) * 2 + half) * 128
                            nc.tensor.matmul(
                                ps[:],
                                w1_sb[:, col:col + 128],
                                xt[k][:],
                                start=(k == 0),
                                stop=(k == DK - 1),
                            )
                        it_ps[plane][half] = ps

                for half in range(NH):
                    cs = cos_sb[:, half * L + ccol:half * L + ccol + 512]
                    sn = sin_sb[:, half * L + ccol:half * L + ccol + 512]
                    # stage PSUM -> SBUF on the (otherwise idle) scalar engine
                    ure = uv_p.tile([128, 512], EW_DT, tag="uv", name=f"ure_{rc}_{half}")
                    nc.scalar.activation(ure[:], it_ps[0][half][:], ACT_COPY)
                    uim = uv_p.tile([128, 512], EW_DT, tag="uv", name=f"uim_{rc}_{half}")
                    nc.scalar.activation(uim[:], it_ps[1][half][:], ACT_COPY)
                    # v = e^{-i theta l} * u
                    t1 = uv_p.tile([128, 512], EW_DT, tag="uv", name=f"t1_{rc}_{half}")
                    nc.vector.tensor_mul(t1[:], ure[:], cs)
                    t2 = uv_p.tile([128, 512], EW_DT, tag="uv", name=f"t2_{rc}_{half}")
                    nc.vector.tensor_mul(t2[:], uim[:], sn)
                    t3 = uv_p.tile([128, 512], EW_DT, tag="uv", name=f"t3_{rc}_{half}")
                    nc.vector.tensor_mul(t3[:], uim[:], cs)
                    t4 = uv_p.tile([128, 512], EW_DT, tag="uv", name=f"t4_{rc}_{half}")
                    nc.gpsimd.tensor_mul(t4[:], ure[:], sn)
                    vre = uv_p.tile([128, 512], EW_DT, tag="uv", name=f"vre_{rc}_{half}")
                    nc.vector.tensor_add(vre[:], t1[:], t2[:])
                    vim = uv_p.tile([128, 512], EW_DT, tag="uv", name=f"vim_{rc}_{half}")
                    nc.gpsimd.tensor_sub(vim[:], t3[:], t4[:])

                    # chained scans for this chunk (reset at batch start)
                    rho_b = _bcast_cols(rho_sb[:, half:half + 1], 512)
                    for plane, vch in ((0, vre), (1, vim)):
                        wp = w_pl[plane][half]
                        if rc % CPB == 0:
                            init = 0.0
                        else:
                            init = wp[:, rc * 512 - 1:rc * 512]
                        nc.vector.tensor_tensor_scan(
                            out=wp[:, dst],
                            data0=rho_b,
                            data1=vch[:],
                            initial=init,
                            op0=AluOp.mult,
                            op1=AluOp.add,
                        )

                    # s = e^{+i theta l} * w for this chunk
                    wre = w_pl[0][half][:, dst]
                    wim = w_pl[1][half][:, dst]
                    q1 = uv_p.tile([128, 512], EW_DT, tag="uv", name=f"q1_{rc}_{half}")
                    nc.vector.tensor_mul(q1[:], wre, cs)
                    q2 = uv_p.tile([128, 512], EW_DT, tag="uv", name=f"q2_{rc}_{half}")
                    nc.vector.tensor_mul(q2[:], wim, sn)
                    q3 = uv_p.tile([128, 512], EW_DT, tag="uv", name=f"q3_{rc}_{half}")
                    nc.vector.tensor_mul(q3[:], wim, cs)
                    q4 = uv_p.tile([128, 512], EW_DT, tag="uv", name=f"q4_{rc}_{half}")
                    nc.gpsimd.tensor_mul(q4[:], wre, sn)
                    sre = s_p.tile([128, 512], S_DT, tag="sch",
                                   name=f"sre_{rc}_{half}")
                    nc.vector.tensor_sub(sre[:], q1[:], q2[:])
                    sim = s_p.tile([128, 512], S_DT, tag="sch",
                                   name=f"sim_{rc}_{half}")
                    nc.vector.tensor_add(sim[:], q3[:], q4[:])
                    if half == 0:
                        s_ch = [[None] * NH, [None] * NH]
                    s_ch[0][half] = sre
                    s_ch[1][half] = sim

                # output tiles for this chunk
                for rt2 in range(4):
                    rt = rc * 4 + rt2
                    ps = ps_y.tile([128, D], F32, tag="y", name=f"ys_{rt}")
                    first = True
                    for plane in range(2):
                        for half in range(NH):
                            nc.tensor.matmul(
                                ps[:],
                                s_ch[plane][half][:, rt2 * 128:(rt2 + 1) * 128],
                                w2_sb[:, (plane * 2 + half) * D:(plane * 2 + half + 1) * D],
                                start=first,
                                stop=(plane == 1 and half == NH - 1),
                            )
                            first = False
                    ysb = y_p.tile([128, D], F32, tag="ysb", name=f"ysb_{rt}")
                    nc.scalar.activation(
                        ysb[:], ps[:], ACT_COPY,
                        scale=mask_sb[:, rt:rt + 1],
                    )
                    nc.gpsimd.dma_start(
                        ysb[:], xadd_d.ap()[rt * 128:(rt + 1) * 128, :],
                        accum_op=AluOp.add,
                    )
                    nc.sync.dma_start(y_d.ap()[rt * 128:(rt + 1) * 128, :], ysb[:])

    nc.compile()
    return nc


_NC_CACHE = {}


def _get_nc():
    if "nc" not in _NC_CACHE:
        _NC_CACHE["nc"] = build_nc()
    return _NC_CACHE["nc"]


def _host_prep(lengths, lambda_real_log, lambda_imag, log_dt, B_re, B_im, C_re, C_im):
    lam_re = -np.exp(np.asarray(lambda_real_log, np.float64))
    lam_im = np.asarray(lambda_imag, np.float64)
    dtv = np.log1p(np.exp(np.float64(log_dt))) + 1e-4
    rho = np.exp(dtv * lam_re)                       # [N]
    theta = dtv * lam_im                             # [N]
    lam = lam_re + 1j * lam_im
    abar = np.exp(dtv * lam)
    bb = ((abar - 1.0) / lam)[:, None] * (
        np.asarray(B_re, np.float64) + 1j * np.asarray(B_im, np.float64)
    )                                                # [N, D] complex
    bb_planes = (np.ascontiguousarray(bb.real), np.ascontiguousarray(bb.imag))

    w1 = np.empty((128, DK * 2 * NH * 128), np.float32)
    for k in range(DK):
        for plane in range(2):
            for half in range(NH):
                col = ((k * 2 + plane) * 2 + half) * 128
                w1[:, col:col + 128] = bb_planes[plane][
                    half * 128:(half + 1) * 128, k * 128:(k + 1) * 128
                ].T.astype(np.float32)

    import ml_dtypes as _mld
    w2 = np.empty((128, 2 * NH * D), np.dtype(_mld.bfloat16))
    c_planes = (np.asarray(C_re, np.float64), -np.asarray(C_im, np.float64))  # [D, N]
    for plane in range(2):
        for half in range(NH):
            col = (plane * 2 + half) * D
            w2[:, col:col + D] = c_planes[plane][
                :, half * 128:(half + 1) * 128
            ].T.astype(np.float32)

    import ml_dtypes
    bf16 = np.dtype(ml_dtypes.bfloat16)
    l_idx = np.arange(L, dtype=np.float64)
    cosst = np.empty((128, NH * L), bf16)
    sinst = np.empty((128, NH * L), bf16)
    for half in range(NH):
        ph = theta[half * 128:(half + 1) * 128, None] * l_idx[None, :]
        cosst[:, half * L:(half + 1) * L] = np.cos(ph).astype(bf16)
        sinst[:, half * L:(half + 1) * L] = np.sin(ph).astype(bf16)

    rho_in = np.empty((128, NH), np.float32)
    for half in range(NH):
        rho_in[:, half] = rho[half * 128:(half + 1) * 128]

    mask_bl = (np.arange(L)[None, :] < np.asarray(lengths)[:, None]).astype(np.float32)  # [B, L]
    return w1, w2, cosst, sinst, rho_in, mask_bl


def _make_in_maps(x, xadd, w1, w2, cosst, sinst, rho_in, mask_bl):
    in_maps = []
    for c in range(NCORES):
        bsl = slice(c * BL, (c + 1) * BL)
        maskc = np.ascontiguousarray(mask_bl[bsl].reshape(R).reshape(RT, 128).T)
        xt = np.ascontiguousarray(x[bsl].reshape(R, D).T)
        in_maps.append({
            "xt": xt,
            "xadd": np.ascontiguousarray(xadd[bsl].reshape(R, D)),
            "w1": w1, "w2": w2, "cost": cosst, "sint": sinst,
            "rho": rho_in, "maskc": maskc,
        })
    return in_maps


def kernel(x, lengths, lambda_real_log, lambda_imag, log_dt, B_re, B_im, C_re, C_im,
           D_weight):
    x = np.asarray(x, np.float32)
    w1, w2, cosst, sinst, rho_in, mask_bl = _host_prep(
        lengths, lambda_real_log, lambda_imag, log_dt, B_re, B_im, C_re, C_im
    )

    Dw = np.asarray(D_weight, np.float32)
    if Dw.shape == (D, D) and np.array_equal(Dw, np.eye(D, dtype=np.float32)):
        xd = x
    else:
        xd = (x.reshape(B * L, D) @ Dw.T.astype(np.float32)).reshape(B, L, D)
    xadd = xd * mask_bl[:, :, None]  # [B, L, D]

    nc = _get_nc()
    in_maps = _make_in_maps(x, xadd, w1, w2, cosst, sinst, rho_in, mask_bl)

    last_err = None
    for _ in range(3):  # device errors are occasionally transient under axon
        try:
            res = run_bass_kernel_spmd(nc, in_maps, core_ids=list(range(NCORES)))
            break
        except Exception as e:  # noqa: BLE001
            last_err = e
    else:
        raise last_err
    y = np.empty((B, L, D), np.float32)
    for c in range(NCORES):
        y[c * BL:(c + 1) * BL] = res.results[c]["y"].reshape(BL, L, D)
    return y


# revision 20
# speedup vs baseline: 1.0770x; 1.0770x over previous
"""Trainium2 Bass kernel for DiagonalS5SSM.

Math (per batch b, with the reference's where(valid,...) elided — valid is a
prefix mask in l and the output is masked by the same prefix, so the frozen
tail states never reach the output):

    it[l, n]  = sum_d x[b, l, d] * bbar[n, d]          (complex)
    s[l, n]   = abar[n] * s[l-1, n] + it[l, n]         (complex scan)
    y[b, l, :] = mask[l] * (Re(s[l] @ c^T) + x[b, l] @ D^T)

The complex scan is decoupled into two real scans via polar form
abar = rho * e^{i theta} (rho < 1, so no dynamic-range blowup):

    v[l] = e^{-i theta l} * it[l]       (elementwise rotation)
    w[l] = rho * w[l-1] + v[l]          (HW tensor_tensor_scan per plane)
    s[l] = e^{+i theta l} * w[l]        (rotation back)

Sharding: data-parallel over batch, 2 batches per core, no collectives.
Matmuls run as float32r (TF32-like, 1 cyc/row on the PE at free>=256).
x is pre-transposed on the host so the contraction dim lands on partitions
without PE transposes. The whole pipeline is chunk-streamed in 512-row
chunks; the scan chains across chunks via initial=prev[:, -1:].
"""

import numpy as np

import concourse.bass as bass
import concourse.tile as tile
from concourse import bacc, mybir
from concourse.bass_utils import run_bass_kernel_spmd

B, L, D, N = 16, 2048, 512, 256
NCORES = 8
BL = B // NCORES          # batches per core
R = BL * L                # rows per core (r = b*L + l)
NH = N // 128             # n-halves
DK = D // 128             # d-chunks
RT = R // 128             # 128-row tiles per core
RCH = R // 512            # 512-row chunks per core
CPB = L // 512            # chunks per batch

F32 = mybir.dt.float32
BF16 = mybir.dt.bfloat16
MM_DT = mybir.dt.float32r  # matmul operand dtype (float32r: 1 cyc/row)
EW_DT = BF16               # elementwise/rotation dtype (2x DVE mode)
S_DT = BF16                # s-plane / stage-5 matmul dtype

AluOp = mybir.AluOpType
ACT_COPY = mybir.ActivationFunctionType.Copy


def _bcast_cols(ap: bass.AP, n: int) -> bass.AP:
    """[128, 1] AP -> [128, n] free-broadcast (step-0) AP."""
    return bass.AP(tensor=ap.tensor, offset=ap.offset, ap=[ap.ap[0], [0, n]])


def build_nc():
    nc = bacc.Bacc(
        "TRN2",
        target_bir_lowering=False,
        debug=False,
        enable_asserts=False,
        num_devices=NCORES,
    )

    xt_d = nc.dram_tensor("xt", [D, R], MM_DT, kind="ExternalInput")
    xadd_d = nc.dram_tensor("xadd", [R, D], F32, kind="ExternalInput")
    w1_d = nc.dram_tensor("w1", [128, DK * 2 * NH * 128], MM_DT, kind="ExternalInput")
    w2_d = nc.dram_tensor("w2", [128, 2 * NH * D], S_DT, kind="ExternalInput")
    cos_d = nc.dram_tensor("cost", [128, NH * L], EW_DT, kind="ExternalInput")
    sin_d = nc.dram_tensor("sint", [128, NH * L], EW_DT, kind="ExternalInput")
    rho_d = nc.dram_tensor("rho", [128, NH], F32, kind="ExternalInput")
    mask_d = nc.dram_tensor("maskc", [128, RT], F32, kind="ExternalInput")
    y_d = nc.dram_tensor("y", [R, D], F32, kind="ExternalOutput")

    with tile.TileContext(nc) as tc:
        with (
            tc.tile_pool(name="consts", bufs=1) as consts,
            tc.tile_pool(name="wplanes", bufs=4) as wplanes,
            tc.tile_pool(name="xtp", bufs=6) as xt_p,
            tc.tile_pool(name="uvp", bufs=10) as uv_p,
            tc.tile_pool(name="sp", bufs=6) as s_p,
            tc.tile_pool(name="yp", bufs=3) as y_p,
            tc.tile_pool(name="ps_it", bufs=5, space="PSUM") as ps_it,
            tc.tile_pool(name="ps_y", bufs=3, space="PSUM") as ps_y,
        ):
            w1_sb = consts.tile([128, DK * 2 * NH * 128], MM_DT, tag="w1")
            for q in range(4):
                nc.sync.dma_start(w1_sb[q * 32:(q + 1) * 32, :],
                                  w1_d.ap()[q * 32:(q + 1) * 32, :])
            w2_sb = consts.tile([128, 2 * NH * D], S_DT, tag="w2")
            for q in range(2):
                nc.sync.dma_start(w2_sb[q * 64:(q + 1) * 64, :],
                                  w2_d.ap()[q * 64:(q + 1) * 64, :])
            cos_sb = consts.tile([128, NH * L], EW_DT, tag="cos")
            nc.gpsimd.dma_start(cos_sb[:], cos_d.ap())
            sin_sb = consts.tile([128, NH * L], EW_DT, tag="sin")
            nc.gpsimd.dma_start(sin_sb[:], sin_d.ap())
            rho_sb = consts.tile([128, NH], F32, tag="rho")
            nc.gpsimd.dma_start(rho_sb[:], rho_d.ap())
            mask_sb = consts.tile([128, RT], F32, tag="maskc")
            nc.gpsimd.dma_start(mask_sb[:], mask_d.ap())

            # full-width scan outputs, [n-half partitions, (b, l) free]
            w_pl = [
                [wplanes.tile([128, R], EW_DT, tag="wpl", name=f"w_{p}_{h}")
                 for h in range(NH)]
                for p in range(2)
            ]

            for rc in range(RCH):
                ccol = (rc % CPB) * 512   # l-offset within batch
                dst = slice(rc * 512, rc * 512 + 512)

                # x^T chunk [d, r] straight from DRAM (host pre-transposed)
                xt = []
                for k in range(DK):
                    t = xt_p.tile([128, 512], MM_DT, tag="xt", name=f"xt_{rc}_{k}")
                    nc.sync.dma_start(
                        t[:64, :], xt_d.ap()[k * 128:k * 128 + 64, dst]
                    )
                    nc.sync.dma_start(
                        t[64:, :], xt_d.ap()[k * 128 + 64:(k + 1) * 128, dst]
                    )
                    xt.append(t)

                # stage 1: it[n, r-chunk] = sum_d bbar[n, d] x[r, d]
                it_ps = [[None] * NH for _ in range(2)]
                for plane in range(2):
                    for half in range(NH):
                        ps = ps_it.tile([128, 512], F32, tag="it",
                                        name=f"it_{rc}_{plane}_{half}")
                        for k in range(DK):
                            col = ((k * 2 + plane) * 2 + half) * 128
                            nc.tensor.matmul(
                                ps[:],
                                w1_sb[:, col:col + 128],
                                xt[k][:],
                                start=(k == 0),
                                stop=(k == DK - 1),
                            )
                        it_ps[plane][half] = ps

                for half in range(NH):
                    cs = cos_sb[:, half * L + ccol:half * L + ccol + 512]
                    sn = sin_sb[:, half * L + ccol:half * L + ccol + 512]
                    # stage PSUM -> SBUF on the (otherwise idle) scalar engine
                    ure = uv_p.tile([128, 512], EW_DT, tag="uv", name=f"ure_{rc}_{half}")
                    nc.scalar.activation(ure[:], it_ps[0][half][:], ACT_COPY)
                    uim = uv_p.tile([128, 512], EW_DT, tag="uv", name=f"uim_{rc}_{half}")
                    nc.scalar.activation(uim[:], it_ps[1][half][:], ACT_COPY)
                    # v = e^{-i theta l} * u
                    t1 = uv_p.tile([128, 512], EW_DT, tag="uv", name=f"t1_{rc}_{half}")
                    nc.vector.tensor_mul(t1[:], ure[:], cs)
                    t2 = uv_p.tile([128, 512], EW_DT, tag="uv", name=f"t2_{rc}_{half}")
                    nc.vector.tensor_mul(t2[:], uim[:], sn)
                    t3 = uv_p.tile([128, 512], EW_DT, tag="uv", name=f"t3_{rc}_{half}")
                    nc.vector.tensor_mul(t3[:], uim[:], cs)
                    t4 = uv_p.tile([128, 512], EW_DT, tag="uv", name=f"t4_{rc}_{half}")
                    nc.vector.tensor_mul(t4[:], ure[:], sn)
                    vre = uv_p.tile([128, 512], EW_DT, tag="uv", name=f"vre_{rc}_{half}")
                    nc.vector.tensor_add(vre[:], t1[:], t2[:])
                    vim = uv_p.tile([128, 512], EW_DT, tag="uv", name=f"vim_{rc}_{half}")
                    nc.vector.tensor_sub(vim[:], t3[:], t4[:])

                    # chained scans for this chunk (reset at batch start)
                    rho_b = _bcast_cols(rho_sb[:, half:half + 1], 512)
                    for plane, vch in ((0, vre), (1, vim)):
                        wp = w_pl[plane][half]
                        if rc % CPB == 0:
                            init = 0.0
                        else:
                            init = wp[:, rc * 512 - 1:rc * 512]
                        nc.vector.tensor_tensor_scan(
                            out=wp[:, dst],
                            data0=rho_b,
                            data1=vch[:],
                            initial=init,
                            op0=AluOp.mult,
                            op1=AluOp.add,
                        )

                    # s = e^{+i theta l} * w for this chunk
                    wre = w_pl[0][half][:, dst]
                    wim = w_pl[1][half][:, dst]
                    q1 = uv_p.tile([128, 512], EW_DT, tag="uv", name=f"q1_{rc}_{half}")
                    nc.vector.tensor_mul(q1[:], wre, cs)
                    q2 = uv_p.tile([128, 512], EW_DT, tag="uv", name=f"q2_{rc}_{half}")
                    nc.vector.tensor_mul(q2[:], wim, sn)
                    q3 = uv_p.tile([128, 512], EW_DT, tag="uv", name=f"q3_{rc}_{half}")
                    nc.vector.tensor_mul(q3[:], wim, cs)
                    q4 = uv_p.tile([128, 512], EW_DT, tag="uv", name=f"q4_{rc}_{half}")
                    nc.vector.tensor_mul(q4[:], wre, sn)
                    sre = s_p.tile([128, 512], S_DT, tag="sch",
                                   name=f"sre_{rc}_{half}")
                    nc.vector.tensor_sub(sre[:], q1[:], q2[:])
                    sim = s_p.tile([128, 512], S_DT, tag="sch",
                                   name=f"sim_{rc}_{half}")
                    nc.vector.tensor_add(sim[:], q3[:], q4[:])
                    if half == 0:
                        s_ch = [[None] * NH, [None] * NH]
                    s_ch[0][half] = sre
                    s_ch[1][half] = sim

                # output tiles for this chunk
                for rt2 in range(4):
                    rt = rc * 4 + rt2
                    ps = ps_y.tile([128, D], F32, tag="y", name=f"ys_{rt}")
                    first = True
                    for plane in range(2):
                        for half in range(NH):
                            nc.tensor.matmul(
                                ps[:],
                                s_ch[plane][half][:, rt2 * 128:(rt2 + 1) * 128],
                                w2_sb[:, (plane * 2 + half) * D:(plane * 2 + half + 1) * D],
                                start=first,
                                stop=(plane == 1 and half == NH - 1),
                            )
                            first = False
                    ysb = y_p.tile([128, D], F32, tag="ysb", name=f"ysb_{rt}")
                    nc.scalar.activation(
                        ysb[:], ps[:], ACT_COPY,
                        scale=mask_sb[:, rt:rt + 1],
                    )
                    nc.gpsimd.dma_start(
                        ysb[:], xadd_d.ap()[rt * 128:(rt + 1) * 128, :],
                        accum_op=AluOp.add,
                    )
                    nc.sync.dma_start(y_d.ap()[rt * 128:(rt + 1) * 128, :], ysb[:])

    nc.compile()
    return nc


_NC_CACHE = {}


def _get_nc():
    if "nc" not in _NC_CACHE:
        _NC_CACHE["nc"] = build_nc()
    return _NC_CACHE["nc"]


def _host_prep(lengths, lambda_real_log, lambda_imag, log_dt, B_re, B_im, C_re, C_im):
    lam_re = -np.exp(np.asarray(lambda_real_log, np.float64))
    lam_im = np.asarray(lambda_imag, np.float64)
    dtv = np.log1p(np.exp(np.float64(log_dt))) + 1e-4
    rho = np.exp(dtv * lam_re)                       # [N]
    theta = dtv * lam_im                             # [N]
    lam = lam_re + 1j * lam_im
    abar = np.exp(dtv * lam)
    bb = ((abar - 1.0) / lam)[:, None] * (
        np.asarray(B_re, np.float64) + 1j * np.asarray(B_im, np.float64)
    )                                                # [N, D] complex
    bb_planes = (np.ascontiguousarray(bb.real), np.ascontiguousarray(bb.imag))

    w1 = np.empty((128, DK * 2 * NH * 128), np.float32)
    for k in range(DK):
        for plane in range(2):
            for half in range(NH):
                col = ((k * 2 + plane) * 2 + half) * 128
                w1[:, col:col + 128] = bb_planes[plane][
                    half * 128:(half + 1) * 128, k * 128:(k + 1) * 128
                ].T.astype(np.float32)

    import ml_dtypes as _mld
    w2 = np.empty((128, 2 * NH * D), np.dtype(_mld.bfloat16))
    c_planes = (np.asarray(C_re, np.float64), -np.asarray(C_im, np.float64))  # [D, N]
    for plane in range(2):
        for half in range(NH):
            col = (plane * 2 + half) * D
            w2[:, col:col + D] = c_planes[plane][
                :, half * 128:(half + 1) * 128
            ].T.astype(np.float32)

    import ml_dtypes
    bf16 = np.dtype(ml_dtypes.bfloat16)
    l_idx = np.arange(L, dtype=np.float64)
    cosst = np.empty((128, NH * L), bf16)
    sinst = np.empty((128, NH * L), bf16)
    for half in range(NH):
        ph = theta[half * 128:(half + 1) * 128, None] * l_idx[None, :]
        cosst[:, half * L:(half + 1) * L] = np.cos(ph).astype(bf16)
        sinst[:, half * L:(half + 1) * L] = np.sin(ph).astype(bf16)

    rho_in = np.empty((128, NH), np.float32)
    for half in range(NH):
        rho_in[:, half] = rho[half * 128:(half + 1) * 128]

    mask_bl = (np.arange(L)[None, :] < np.asarray(lengths)[:, None]).astype(np.float32)  # [B, L]
    return w1, w2, cosst, sinst, rho_in, mask_bl


def _make_in_maps(x, xadd, w1, w2, cosst, sinst, rho_in, mask_bl):
    in_maps = []
    for c in range(NCORES):
        bsl = slice(c * BL, (c + 1) * BL)
        maskc = np.ascontiguousarray(mask_bl[bsl].reshape(R).reshape(RT, 128).T)
        xt = np.ascontiguousarray(x[bsl].reshape(R, D).T)
        in_maps.append({
            "xt": xt,
            "xadd": np.ascontiguousarray(xadd[bsl].reshape(R, D)),
            "w1": w1, "w2": w2, "cost": cosst, "sint": sinst,
            "rho": rho_in, "maskc": maskc,
        })
    return in_maps


def kernel(x, lengths, lambda_real_log, lambda_imag, log_dt, B_re, B_im, C_re, C_im,
           D_weight):
    x = np.asarray(x, np.float32)
    w1, w2, cosst, sinst, rho_in, mask_bl = _host_prep(
        lengths, lambda_real_log, lambda_imag, log_dt, B_re, B_im, C_re, C_im
    )

    Dw = np.asarray(D_weight, np.float32)
    if Dw.shape == (D, D) and np.array_equal(Dw, np.eye(D, dtype=np.float32)):
        xd = x
    else:
        xd = (x.reshape(B * L, D) @ Dw.T.astype(np.float32)).reshape(B, L, D)
    xadd = xd * mask_bl[:, :, None]  # [B, L, D]

    nc = _get_nc()
    in_maps = _make_in_maps(x, xadd, w1, w2, cosst, sinst, rho_in, mask_bl)

    last_err = None
    for _ in range(3):  # device errors are occasionally transient under axon
        try:
            res = run_bass_kernel_spmd(nc, in_maps, core_ids=list(range(NCORES)))
            break
        except Exception as e:  # noqa: BLE001
            last_err = e
    else:
        raise last_err
    y = np.empty((B, L, D), np.float32)
    for c in range(NCORES):
        y[c * BL:(c + 1) * BL] = res.results[c]["y"].reshape(BL, L, D)
    return y


# revision 21
# speedup vs baseline: 1.1797x; 1.0954x over previous
"""Trainium2 Bass kernel for DiagonalS5SSM.

Math (per batch b, with the reference's where(valid,...) elided — valid is a
prefix mask in l and the output is masked by the same prefix, so the frozen
tail states never reach the output):

    it[l, n]  = sum_d x[b, l, d] * bbar[n, d]          (complex)
    s[l, n]   = abar[n] * s[l-1, n] + it[l, n]         (complex scan)
    y[b, l, :] = mask[l] * (Re(s[l] @ c^T) + x[b, l] @ D^T)

The complex scan is decoupled into two real scans via polar form
abar = rho * e^{i theta} (rho < 1, so no dynamic-range blowup):

    v[l] = e^{-i theta l} * it[l]       (elementwise rotation)
    w[l] = rho * w[l-1] + v[l]          (HW tensor_tensor_scan per plane)
    s[l] = e^{+i theta l} * w[l]        (rotation back)

Sharding: data-parallel over batch, 2 batches per core, no collectives.
Matmuls run as float32r (TF32-like, 1 cyc/row on the PE at free>=256).
x is pre-transposed on the host so the contraction dim lands on partitions
without PE transposes. The whole pipeline is chunk-streamed in 512-row
chunks; the scan chains across chunks via initial=prev[:, -1:].
"""

import numpy as np

import concourse.bass as bass
import concourse.tile as tile
from concourse import bacc, mybir
from concourse.bass_utils import run_bass_kernel_spmd

B, L, D, N = 16, 2048, 512, 256
NCORES = 8
BL = B // NCORES          # batches per core
R = BL * L                # rows per core (r = b*L + l)
NH = N // 128             # n-halves
DK = D // 128             # d-chunks
RT = R // 128             # 128-row tiles per core
RCH = R // 512            # 512-row chunks per core
CPB = L // 512            # chunks per batch

F32 = mybir.dt.float32
BF16 = mybir.dt.bfloat16
MM_DT = mybir.dt.float32r  # matmul operand dtype (float32r: 1 cyc/row)
EW_DT = BF16               # elementwise/rotation dtype (2x DVE mode)
S_DT = BF16                # s-plane / stage-5 matmul dtype

AluOp = mybir.AluOpType
ACT_COPY = mybir.ActivationFunctionType.Copy


def _bcast_cols(ap: bass.AP, n: int) -> bass.AP:
    """[128, 1] AP -> [128, n] free-broadcast (step-0) AP."""
    return bass.AP(tensor=ap.tensor, offset=ap.offset, ap=[ap.ap[0], [0, n]])


def build_nc():
    nc = bacc.Bacc(
        "TRN2",
        target_bir_lowering=False,
        debug=False,
        enable_asserts=False,
        num_devices=NCORES,
    )

    xt_d = nc.dram_tensor("xt", [D, R], MM_DT, kind="ExternalInput")
    xadd_d = nc.dram_tensor("xadd", [R, D], F32, kind="ExternalInput")
    w1_d = nc.dram_tensor("w1", [128, DK * 2 * NH * 128], MM_DT, kind="ExternalInput")
    w2_d = nc.dram_tensor("w2", [128, 2 * NH * D], S_DT, kind="ExternalInput")
    cos_d = nc.dram_tensor("cost", [128, NH * L], EW_DT, kind="ExternalInput")
    sin_d = nc.dram_tensor("sint", [128, NH * L], EW_DT, kind="ExternalInput")
    rho_d = nc.dram_tensor("rho", [128, NH], F32, kind="ExternalInput")
    mask_d = nc.dram_tensor("maskc", [128, RT], F32, kind="ExternalInput")
    y_d = nc.dram_tensor("y", [R, D], F32, kind="ExternalOutput")

    with tile.TileContext(nc) as tc:
        with (
            tc.tile_pool(name="consts", bufs=1) as consts,
            tc.tile_pool(name="wplanes", bufs=4) as wplanes,
            tc.tile_pool(name="xtp", bufs=10) as xt_p,
            tc.tile_pool(name="uvp", bufs=14) as uv_p,
            tc.tile_pool(name="sp", bufs=6) as s_p,
            tc.tile_pool(name="yp", bufs=3) as y_p,
            tc.tile_pool(name="ps_it", bufs=5, space="PSUM") as ps_it,
            tc.tile_pool(name="ps_y", bufs=3, space="PSUM") as ps_y,
        ):
            w1_sb = []
            for k in range(DK):
                w1k = consts.tile([128, 2 * NH * 128], MM_DT, tag=f"w1_{k}",
                                  name=f"w1sb_{k}")
                for q in range(2):
                    nc.sync.dma_start(
                        w1k[q * 64:(q + 1) * 64, :],
                        w1_d.ap()[q * 64:(q + 1) * 64, k * 512:(k + 1) * 512],
                    )
                w1_sb.append(w1k)
            w2_sb = consts.tile([128, 2 * NH * D], S_DT, tag="w2")
            for q in range(2):
                nc.sync.dma_start(w2_sb[q * 64:(q + 1) * 64, :],
                                  w2_d.ap()[q * 64:(q + 1) * 64, :])
            cos_sb = consts.tile([128, NH * L], EW_DT, tag="cos")
            nc.gpsimd.dma_start(cos_sb[:], cos_d.ap())
            sin_sb = consts.tile([128, NH * L], EW_DT, tag="sin")
            nc.gpsimd.dma_start(sin_sb[:], sin_d.ap())
            rho_sb = consts.tile([128, NH], F32, tag="rho")
            nc.gpsimd.dma_start(rho_sb[:], rho_d.ap())
            mask_sb = consts.tile([128, RT], F32, tag="maskc")
            nc.gpsimd.dma_start(mask_sb[:], mask_d.ap())

            # full-width scan outputs, [n-half partitions, (b, l) free]
            w_pl = [
                [wplanes.tile([128, R], EW_DT, tag="wpl", name=f"w_{p}_{h}")
                 for h in range(NH)]
                for p in range(2)
            ]

            rc_order = [0, 4, 1, 5, 2, 6, 3, 7]
            for rc in rc_order:
                ccol = (rc % CPB) * 512   # l-offset within batch
                dst = slice(rc * 512, rc * 512 + 512)

                # x^T chunk [d, r] straight from DRAM (host pre-transposed)
                xt = []
                for k in range(DK):
                    t = xt_p.tile([128, 512], MM_DT, tag="xt", name=f"xt_{rc}_{k}")
                    nc.sync.dma_start(
                        t[:64, :], xt_d.ap()[k * 128:k * 128 + 64, dst]
                    )
                    nc.sync.dma_start(
                        t[64:, :], xt_d.ap()[k * 128 + 64:(k + 1) * 128, dst]
                    )
                    xt.append(t)

                # stage 1: it[n, r-chunk] = sum_d bbar[n, d] x[r, d]
                it_ps = [[None] * NH for _ in range(2)]
                for plane in range(2):
                    for half in range(NH):
                        ps = ps_it.tile([128, 512], F32, tag="it",
                                        name=f"it_{rc}_{plane}_{half}")
                        for k in range(DK):
                            col = (plane * 2 + half) * 128
                            nc.tensor.matmul(
                                ps[:],
                                w1_sb[k][:, col:col + 128],
                                xt[k][:],
                                start=(k == 0),
                                stop=(k == DK - 1),
                            )
                        it_ps[plane][half] = ps

                for half in range(NH):
                    cs = cos_sb[:, half * L + ccol:half * L + ccol + 512]
                    sn = sin_sb[:, half * L + ccol:half * L + ccol + 512]
                    # stage PSUM -> SBUF on the (otherwise idle) scalar engine
                    ure = uv_p.tile([128, 512], EW_DT, tag="uv", name=f"ure_{rc}_{half}")
                    nc.scalar.activation(ure[:], it_ps[0][half][:], ACT_COPY)
                    uim = uv_p.tile([128, 512], EW_DT, tag="uv", name=f"uim_{rc}_{half}")
                    nc.scalar.activation(uim[:], it_ps[1][half][:], ACT_COPY)
                    # v = e^{-i theta l} * u
                    t1 = uv_p.tile([128, 512], EW_DT, tag="uv", name=f"t1_{rc}_{half}")
                    nc.vector.tensor_mul(t1[:], ure[:], cs)
                    t2 = uv_p.tile([128, 512], EW_DT, tag="uv", name=f"t2_{rc}_{half}")
                    nc.vector.tensor_mul(t2[:], uim[:], sn)
                    t3 = uv_p.tile([128, 512], EW_DT, tag="uv", name=f"t3_{rc}_{half}")
                    nc.vector.tensor_mul(t3[:], uim[:], cs)
                    t4 = uv_p.tile([128, 512], EW_DT, tag="uv", name=f"t4_{rc}_{half}")
                    nc.vector.tensor_mul(t4[:], ure[:], sn)
                    vre = uv_p.tile([128, 512], EW_DT, tag="uv", name=f"vre_{rc}_{half}")
                    nc.vector.tensor_add(vre[:], t1[:], t2[:])
                    vim = uv_p.tile([128, 512], EW_DT, tag="uv", name=f"vim_{rc}_{half}")
                    nc.vector.tensor_sub(vim[:], t3[:], t4[:])

                    # chained scans for this chunk (reset at batch start)
                    rho_b = _bcast_cols(rho_sb[:, half:half + 1], 512)
                    for plane, vch in ((0, vre), (1, vim)):
                        wp = w_pl[plane][half]
                        if rc % CPB == 0:
                            init = 0.0
                        else:
                            init = wp[:, rc * 512 - 1:rc * 512]
                        nc.vector.tensor_tensor_scan(
                            out=wp[:, dst],
                            data0=rho_b,
                            data1=vch[:],
                            initial=init,
                            op0=AluOp.mult,
                            op1=AluOp.add,
                        )

                    # s = e^{+i theta l} * w for this chunk
                    wre = w_pl[0][half][:, dst]
                    wim = w_pl[1][half][:, dst]
                    q1 = uv_p.tile([128, 512], EW_DT, tag="uv", name=f"q1_{rc}_{half}")
                    nc.vector.tensor_mul(q1[:], wre, cs)
                    q2 = uv_p.tile([128, 512], EW_DT, tag="uv", name=f"q2_{rc}_{half}")
                    nc.vector.tensor_mul(q2[:], wim, sn)
                    q3 = uv_p.tile([128, 512], EW_DT, tag="uv", name=f"q3_{rc}_{half}")
                    nc.vector.tensor_mul(q3[:], wim, cs)
                    q4 = uv_p.tile([128, 512], EW_DT, tag="uv", name=f"q4_{rc}_{half}")
                    nc.vector.tensor_mul(q4[:], wre, sn)
                    sre = s_p.tile([128, 512], S_DT, tag="sch",
                                   name=f"sre_{rc}_{half}")
                    nc.vector.tensor_sub(sre[:], q1[:], q2[:])
                    sim = s_p.tile([128, 512], S_DT, tag="sch",
                                   name=f"sim_{rc}_{half}")
                    nc.vector.tensor_add(sim[:], q3[:], q4[:])
                    if half == 0:
                        s_ch = [[None] * NH, [None] * NH]
                    s_ch[0][half] = sre
                    s_ch[1][half] = sim

                # output tiles for this chunk
                for rt2 in range(4):
                    rt = rc * 4 + rt2
                    ps = ps_y.tile([128, D], F32, tag="y", name=f"ys_{rt}")
                    first = True
                    for plane in range(2):
                        for half in range(NH):
                            nc.tensor.matmul(
                                ps[:],
                                s_ch[plane][half][:, rt2 * 128:(rt2 + 1) * 128],
                                w2_sb[:, (plane * 2 + half) * D:(plane * 2 + half + 1) * D],
                                start=first,
                                stop=(plane == 1 and half == NH - 1),
                            )
                            first = False
                    ysb = y_p.tile([128, D], F32, tag="ysb", name=f"ysb_{rt}")
                    nc.scalar.activation(
                        ysb[:], ps[:], ACT_COPY,
                        scale=mask_sb[:, rt:rt + 1],
                    )
                    nc.gpsimd.dma_start(
                        ysb[:], xadd_d.ap()[rt * 128:(rt + 1) * 128, :],
                        accum_op=AluOp.add,
                    )
                    nc.sync.dma_start(y_d.ap()[rt * 128:(rt + 1) * 128, :], ysb[:])

    nc.compile()
    return nc


_NC_CACHE = {}


def _get_nc():
    if "nc" not in _NC_CACHE:
        _NC_CACHE["nc"] = build_nc()
    return _NC_CACHE["nc"]


def _host_prep(lengths, lambda_real_log, lambda_imag, log_dt, B_re, B_im, C_re, C_im):
    lam_re = -np.exp(np.asarray(lambda_real_log, np.float64))
    lam_im = np.asarray(lambda_imag, np.float64)
    dtv = np.log1p(np.exp(np.float64(log_dt))) + 1e-4
    rho = np.exp(dtv * lam_re)                       # [N]
    theta = dtv * lam_im                             # [N]
    lam = lam_re + 1j * lam_im
    abar = np.exp(dtv * lam)
    bb = ((abar - 1.0) / lam)[:, None] * (
        np.asarray(B_re, np.float64) + 1j * np.asarray(B_im, np.float64)
    )                                                # [N, D] complex
    bb_planes = (np.ascontiguousarray(bb.real), np.ascontiguousarray(bb.imag))

    w1 = np.empty((128, DK * 2 * NH * 128), np.float32)
    for k in range(DK):
        for plane in range(2):
            for half in range(NH):
                col = ((k * 2 + plane) * 2 + half) * 128
                w1[:, col:col + 128] = bb_planes[plane][
                    half * 128:(half + 1) * 128, k * 128:(k + 1) * 128
                ].T.astype(np.float32)

    import ml_dtypes as _mld
    w2 = np.empty((128, 2 * NH * D), np.dtype(_mld.bfloat16))
    c_planes = (np.asarray(C_re, np.float64), -np.asarray(C_im, np.float64))  # [D, N]
    for plane in range(2):
        for half in range(NH):
            col = (plane * 2 + half) * D
            w2[:, col:col + D] = c_planes[plane][
                :, half * 128:(half + 1) * 128
            ].T.astype(np.float32)

    import ml_dtypes
    bf16 = np.dtype(ml_dtypes.bfloat16)
    l_idx = np.arange(L, dtype=np.float64)
    cosst = np.empty((128, NH * L), bf16)
    sinst = np.empty((128, NH * L), bf16)
    for half in range(NH):
        ph = theta[half * 128:(half + 1) * 128, None] * l_idx[None, :]
        cosst[:, half * L:(half + 1) * L] = np.cos(ph).astype(bf16)
        sinst[:, half * L:(half + 1) * L] = np.sin(ph).astype(bf16)

    rho_in = np.empty((128, NH), np.float32)
    for half in range(NH):
        rho_in[:, half] = rho[half * 128:(half + 1) * 128]

    mask_bl = (np.arange(L)[None, :] < np.asarray(lengths)[:, None]).astype(np.float32)  # [B, L]
    return w1, w2, cosst, sinst, rho_in, mask_bl


def _make_in_maps(x, xadd, w1, w2, cosst, sinst, rho_in, mask_bl):
    in_maps = []
    for c in range(NCORES):
        bsl = slice(c * BL, (c + 1) * BL)
        maskc = np.ascontiguousarray(mask_bl[bsl].reshape(R).reshape(RT, 128).T)
        xt = np.ascontiguousarray(x[bsl].reshape(R, D).T)
        in_maps.append({
            "xt": xt,
            "xadd": np.ascontiguousarray(xadd[bsl].reshape(R, D)),
            "w1": w1, "w2": w2, "cost": cosst, "sint": sinst,
            "rho": rho_in, "maskc": maskc,
        })
    return in_maps


def kernel(x, lengths, lambda_real_log, lambda_imag, log_dt, B_re, B_im, C_re, C_im,
           D_weight):
    x = np.asarray(x, np.float32)
    w1, w2, cosst, sinst, rho_in, mask_bl = _host_prep(
        lengths, lambda_real_log, lambda_imag, log_dt, B_re, B_im, C_re, C_im
    )

    Dw = np.asarray(D_weight, np.float32)
    if Dw.shape == (D, D) and np.array_equal(Dw, np.eye(D, dtype=np.float32)):
        xd = x
    else:
        xd = (x.reshape(B * L, D) @ Dw.T.astype(np.float32)).reshape(B, L, D)
    xadd = xd * mask_bl[:, :, None]  # [B, L, D]

    nc = _get_nc()
    in_maps = _make_in_maps(x, xadd, w1, w2, cosst, sinst, rho_in, mask_bl)

    last_err = None
    for _ in range(3):  # device errors are occasionally transient under axon
        try:
            res = run_bass_kernel_spmd(nc, in_maps, core_ids=list(range(NCORES)))
            break
        except Exception as e:  # noqa: BLE001
            last_err = e
    else:
        raise last_err
    y = np.empty((B, L, D), np.float32)
    for c in range(NCORES):
        y[c * BL:(c + 1) * BL] = res.results[c]["y"].reshape(BL, L, D)
    return y
